# revision 26
# baseline (speedup 1.0000x reference)
"""Trainium2 Bass kernel for nn_DoubleLSTM: 2-layer stacked LSTM (Keras gate
order) + sigmoid dense head.

Shapes (hardcoded): B=256, T=2048, D=32, H=64.  8 NeuronCores, data-parallel:
core c processes batch rows [c*32, (c+1)*32).

Per-core layout (Bc = 32 batch rows per core), v2:
  - ring1 [96, RING]: slot j = [h1(j-1) (0:64) | x(j) (64:96)], fp16.
  - ring2 [128, RING]: slot j = [h1(j) (0:64) | h2(j-1) (64:128)], fp16.
  - Per layer, gates computed as two [*,128] matmul strips into PSUM:
      strip a = [i; f], strip b = [2g; o]  (g columns pre-scaled by 2 so a
      single sigmoid activation covers all four gates: tanh(g) = 2*sig(2g)-1).
  - L1 critical chain per step: MM pair -> sigmoid -> E=i*sg -> D=2E-i ->
    cc=D+A (A=f*c computed on Pool in parallel) -> tanh -> h1=o*tanh.
    L2 runs the same chain one phase behind, woven into L1's engine idle
    slots via explicit scheduler floors (tile_wait_until).
  - fp16 sigmoid outputs / tanh outputs / E tiles enable DVE 2x perf mode.
  - Head (y = wd . h2) is computed in 512-col chunks spread across the body
    (PE idle slots), staged to SBUF via Pool copies (keeps the Activation
    engine's table set resident), DMA'd out once per body.
"""

import sys

sys.path.insert(0, "/opt/trn_rl_repo")

import numpy as np

import concourse.bass as bass
import concourse.bacc as bacc
import concourse.tile as tile
from concourse import mybir
from concourse.bass_utils import run_bass_kernel_spmd

B, T, D, H = 256, 2048, 32, 64
NCORES = 8
BC = B // NCORES          # 32 batch rows per core
SPB = 512                 # steps per body
NBODY = T // SPB          # 16 bodies
RING = SPB * BC           # 4096 ring columns
F32 = mybir.dt.float32
F16 = mybir.dt.float16
SIG = mybir.ActivationFunctionType.Sigmoid
TANH = mybir.ActivationFunctionType.Tanh
MUL = mybir.AluOpType.mult
ADD = mybir.AluOpType.add
SUB = mybir.AluOpType.subtract

PERIOD = 0.00235  # ms; scheduler-floor period per step (ordering hint only)

_CACHE = {}


def build_nc():
    nc = bacc.Bacc("TRN2", target_bir_lowering=False)

    # DRAM I/O. xt is host-pretransposed x: [D, (T+SPB)*BC] (one zero pad body).
    xt = nc.dram_tensor("xt", [D, (NBODY + 1) * RING], F16, kind="ExternalInput")
    v1a = nc.dram_tensor("v1a", [96, 128], F16, kind="ExternalInput")
    v1b = nc.dram_tensor("v1b", [96, 128], F16, kind="ExternalInput")
    v2a = nc.dram_tensor("v2a", [128, 128], F16, kind="ExternalInput")
    v2b = nc.dram_tensor("v2b", [128, 128], F16, kind="ExternalInput")
    wd = nc.dram_tensor("wd", [128, 1], F16, kind="ExternalInput")
    ytb = nc.dram_tensor("ytb", [NBODY + 1, RING], F32, kind="ExternalOutput")

    with tile.TileContext(nc) as tc:
        with (
            tc.tile_pool(name="consts", bufs=1) as consts,
            tc.tile_pool(name="state", bufs=1) as state,
            tc.tile_pool(name="ps", bufs=1, space="PSUM") as psp,
        ):
            # constants
            v1a_t = consts.tile([96, 128], F16)
            v1b_t = consts.tile([96, 128], F16)
            v2a_t = consts.tile([128, 128], F16)
            v2b_t = consts.tile([128, 128], F16)
            wd_t = consts.tile([128, 1], F16)
            for dst, src in (
                (v1a_t, v1a), (v1b_t, v1b), (v2a_t, v2a), (v2b_t, v2b),
                (wd_t, wd),
            ):
                nc.sync.dma_start(dst[:], src[:, :])

            # rings / state
            ring1 = state.tile([96, RING], F16)    # [h1 (0:64); x_t (64:96)]
            ring2 = state.tile([128, RING], F16)   # [h1 (0:64); h2 (64:128)]
            cc1 = state.tile([128, 4 * BC], F32)   # [64:128] slot j%4: c1(j-1)
            cc2 = state.tile([128, 4 * BC], F32)
            s1 = state.tile([128, 4 * BC], F16)    # sig(z1), 2 parity slots
            s2 = state.tile([128, 4 * BC], F16)
            e1 = state.tile([64, BC], F16)         # i*sg
            e2 = state.tile([64, BC], F16)
            d1 = state.tile([64, BC], F32)         # 2E - i
            d2 = state.tile([64, BC], F32)
            b1t = state.tile([64, BC], F32)        # f*c (Pool)
            b2t = state.tile([64, BC], F32)
            tc1 = state.tile([128, BC], F16)       # [64:128] = tanh(c)
            tc2 = state.tile([128, BC], F16)
            yb = state.tile([1, RING], F32)        # head staging

            nc.vector.memset(ring1[:], 0.0)
            nc.vector.memset(ring2[:], 0.0)
            nc.vector.memset(cc1[:], 0.0)
            nc.vector.memset(cc2[:], 0.0)

            # psum: strips side by side, 2 parity slots per layer
            pz1 = psp.tile([128, 4 * BC], F32)
            pz2 = psp.tile([128, 4 * BC], F32)
            hp0 = psp.tile([1, 512], F32)
            hp1 = psp.tile([1, 512], F32)

            # prologue: x block 0
            nc.sync.dma_start(ring1[64:96, :], xt[:, 0:RING])

            def W(ph):
                # scheduler-floor helper: ph in period units
                return tc.tile_wait_until(ph * PERIOD)

            def step(j):
                c = slice(j * BC, (j + 1) * BC)            # ring col slot j
                cn = slice(((j + 1) % SPB) * BC, ((j + 1) % SPB) * BC + BC)
                g = slice((j % 4) * BC, (j % 4) * BC + BC)  # c slot
                gn = slice(((j + 1) % 4) * BC, ((j + 1) % 4) * BC + BC)
                p = (j % 2) * 2 * BC                        # psum/s parity base
                pa = slice(p, p + BC)                       # strip a cols
                pb = slice(p + BC, p + 2 * BC)              # strip b cols
                pab = slice(p, p + 2 * BC)
                base = float(j)

                # ---- layer 1:  [i;f | 2g;o] = [V1a|V1b]^T @ [h1(j-1); x(j)]
                with W(base + 0.00):
                    nc.tensor.matmul(pz1[:, pa], v1a_t[:], ring1[:, c])
                    nc.tensor.matmul(pz1[:, pb], v1b_t[:], ring1[:, c])
                with W(base + 0.28):
                    nc.scalar.activation(s1[:, pab], pz1[:, pab], SIG)
                with W(base + 0.45):
                    nc.gpsimd.tensor_tensor(b1t[:], s1[64:128, pa],
                                            cc1[64:128, g], MUL)   # f*c
                with W(base + 0.46):
                    nc.vector.tensor_tensor(e1[:], s1[0:64, pb],
                                            s1[0:64, pa], MUL)     # E = sg*i
                with W(base + 0.52):
                    nc.vector.scalar_tensor_tensor(
                        d1[:], e1[:], 2.0, s1[0:64, pa], MUL, SUB)  # 2E - i
                with W(base + 0.68):
                    nc.vector.tensor_tensor(cc1[64:128, gn], d1[:],
                                            b1t[:], ADD)           # + f*c
                with W(base + 0.82):
                    nc.scalar.activation(tc1[64:128, :], cc1[64:128, gn], TANH)
                with W(base + 0.97):
                    nc.vector.tensor_tensor(ring1[0:64, cn], s1[64:128, pb],
                                            tc1[64:128, :], MUL)   # h1(j)
                with W(base + 1.07):
                    nc.gpsimd.tensor_copy(ring2[0:64, c], ring1[0:64, cn])

                # ---- layer 2 (phase-lagged; slot c = [h1(j); h2(j-1)])
                with W(base + 1.17):
                    nc.tensor.matmul(pz2[:, pa], v2a_t[:], ring2[:, c])
                    nc.tensor.matmul(pz2[:, pb], v2b_t[:], ring2[:, c])
                with W(base + 1.42):
                    nc.scalar.activation(s2[:, pab], pz2[:, pab], SIG)
                with W(base + 1.46):
                    nc.gpsimd.tensor_tensor(b2t[:], s2[64:128, pa],
                                            cc2[64:128, g], MUL)
                with W(base + 1.50):
                    nc.vector.tensor_tensor(e2[:], s2[0:64, pb],
                                            s2[0:64, pa], MUL)
                with W(base + 1.70):
                    nc.vector.scalar_tensor_tensor(
                        d2[:], e2[:], 2.0, s2[0:64, pa], MUL, SUB)
                with W(base + 1.64):
                    nc.vector.tensor_tensor(cc2[64:128, gn], d2[:],
                                            b2t[:], ADD)
                with W(base + 1.80):
                    nc.scalar.activation(tc2[64:128, :], cc2[64:128, gn], TANH)
                with W(base + 2.11):
                    nc.vector.tensor_tensor(ring2[64:128, cn], s2[64:128, pb],
                                            tc2[64:128, :], MUL)   # h2(j)

            NCHUNK = RING // 512  # 8 head chunks of 512 cols

            with tc.For_i(0, NBODY, 1, hint_engines=(mybir.EngineType.DVE, mybir.EngineType.Activation, mybir.EngineType.PE, mybir.EngineType.Pool, mybir.EngineType.SP)) as iv:
                for j in range(SPB):
                    step(j)
                    # head chunks q=0..6 over slots [16q+1, 16q+16], spread
                    # mid-body into PE idle slots (slot 16q+16 final after
                    # step 16q+15).
                    if j % 16 == 0 and j > 0:
                        q = j // 16 - 1
                        hpq = hp0 if q % 2 == 0 else hp1
                        c0 = (16 * q + 1) * BC
                        with W(float(j) + 0.40):
                            nc.tensor.matmul(hpq[:], wd_t[64:128, :],
                                             ring2[64:128, c0:c0 + 512])
                        with W(float(j) + 1.12):
                            nc.scalar.copy(yb[:, c0:c0 + 256], hpq[:, 0:256])
                        with W(float(j) + 2.12):
                            nc.scalar.copy(yb[:, c0 + 256:c0 + 512],
                                           hpq[:, 256:512])
                # tail: slots [113..127] (480 cols) + slot 0 (32 cols)
                c0 = 497 * BC
                with W(float(SPB) + 0.55):
                    nc.tensor.matmul(hp0[:, 0:480], wd_t[64:128, :],
                                     ring2[64:128, c0:c0 + 480])
                    nc.tensor.matmul(hp1[:, 0:BC], wd_t[64:128, :],
                                     ring2[64:128, 0:BC])
                with W(float(SPB) + 0.85):
                    nc.scalar.copy(yb[:, c0:c0 + 480], hp0[:, 0:480])
                    nc.scalar.copy(yb[:, 0:BC], hp1[:, 0:BC])
                with W(float(SPB) + 1.1):
                    nc.sync.dma_start(ytb[bass.ds(iv, 1), :], yb[:])
                    # prefetch next x block (block NBODY is zero padding)
                    nc.sync.dma_start(
                        ring1[64:96, :], xt[:, bass.ts(iv + 1, RING)])

            # final step's h2 (t = T-1) sits in ring2 slot 0
            nc.tensor.matmul(hp0[0:1, 0:BC], wd_t[64:128, :], ring2[64:128, 0:BC])
            nc.scalar.copy(yb[:, 0:BC], hp0[0:1, 0:BC])
            nc.sync.dma_start(ytb[NBODY : NBODY + 1, 0:BC], yb[:, 0:BC])

    nc.compile()
    return nc


def _prep_inputs(x, W1, U1, b1, W2, U2, b2, Wd):
    """Host-side constant prep (shared across cores) + per-core x transpose."""
    # gate columns already in Keras order i,f,g,o along the 4H axis
    V1 = np.concatenate([U1, W1], axis=0).astype(np.float32)     # [96, 256]
    V2 = np.concatenate([W2, U2], axis=0).astype(np.float32)     # [128, 256]
    # tanh(g) is computed as 2*sigmoid(2g)-1: pre-scale g-gate columns by 2
    V1 = V1.copy(); V2 = V2.copy()
    V1[:, 128:192] *= 2.0
    V2[:, 128:192] *= 2.0
    const = {
        "v1a": np.ascontiguousarray(V1[:, 0:128]).astype(np.float16),
        "v1b": np.ascontiguousarray(V1[:, 128:256]).astype(np.float16),
        "v2a": np.ascontiguousarray(V2[:, 0:128]).astype(np.float16),
        "v2b": np.ascontiguousarray(V2[:, 128:256]).astype(np.float16),
        "wd": np.concatenate(
            [np.zeros((64, 1), np.float16), Wd.astype(np.float16)], axis=0
        ),
    }
    in_maps = []
    for cix in range(NCORES):
        xc = x[cix * BC : (cix + 1) * BC]              # [BC, T, D]
        # -> [D, T, BC] -> [D, T*BC], pad one zero body
        xtc = np.ascontiguousarray(xc.transpose(2, 1, 0)).reshape(D, T * BC).astype(np.float16)
        xtc = np.concatenate([xtc, np.zeros((D, RING), np.float16)], axis=1)
        in_maps.append({"xt": np.ascontiguousarray(xtc), **const})
    return in_maps


def _postprocess(results, bd):
    """ytb [NBODY+1, RING] per core -> y [B, T, 1] with sigmoid + bias."""
    y = np.empty((B, T, 1), np.float32)
    for cix, res in enumerate(results):
        ytb = res["ytb"]                                # [17, 4096]
        body = ytb[:NBODY].reshape(NBODY, SPB, BC)
        # slot j in 1..SPB-1 holds t = k*SPB+j-1; slot 0 holds t = k*SPB+SPB-1
        ytc = np.roll(body, -1, axis=1).reshape(NBODY * SPB, BC)  # [T, BC]
        z = ytc.astype(np.float64) + float(bd[0])
        y[cix * BC : (cix + 1) * BC, :, 0] = (
            1.0 / (1.0 + np.exp(-z))
        ).T.astype(np.float32)
    return y


def _cpu_fallback(x, W1, U1, b1, W2, U2, b2, Wd, bd):
    x = np.asarray(x, np.float32)
    Bn, Tn, _ = x.shape
    Hn = U1.shape[0]
    sig = lambda v: 1 / (1 + np.exp(-v))
    h1 = np.zeros((Bn, Hn), np.float32); c1 = np.zeros((Bn, Hn), np.float32)
    h2 = np.zeros((Bn, Hn), np.float32); c2 = np.zeros((Bn, Hn), np.float32)
    ys = []
    for t in range(Tn):
        z = x[:, t] @ W1 + h1 @ U1 + b1
        i, f, g, o = np.split(z, 4, -1)
        c1 = sig(f) * c1 + sig(i) * np.tanh(g)
        h1 = sig(o) * np.tanh(c1)
        z = h1 @ W2 + h2 @ U2 + b2
        i, f, g, o = np.split(z, 4, -1)
        c2 = sig(f) * c2 + sig(i) * np.tanh(g)
        h2 = sig(o) * np.tanh(c2)
        ys.append(h2)
    hs = np.stack(ys, 1)
    return sig(hs @ Wd + bd).astype(np.float32)


def kernel(x, W1, U1, b1, W2, U2, b2, Wd, bd, **kw):
    if np.any(np.asarray(b1)) or np.any(np.asarray(b2)):
        # device kernel folds zero biases away; rare general case on CPU
        return _cpu_fallback(x, W1, U1, b1, W2, U2, b2, Wd, bd)
    if "nc" not in _CACHE:
        _CACHE["nc"] = build_nc()
    nc = _CACHE["nc"]
    in_maps = _prep_inputs(
        np.asarray(x), np.asarray(W1), np.asarray(U1), np.asarray(b1),
        np.asarray(W2), np.asarray(U2), np.asarray(b2), np.asarray(Wd),
    )
    res = run_bass_kernel_spmd(
        nc, in_maps, core_ids=list(range(NCORES)), **kw
    )
    out = _postprocess(res.results, np.asarray(bd))
    _CACHE["last_result"] = res
    return out
